# revision 1
# baseline (speedup 1.0000x reference)
"""Trainium2 Bass kernel for nn_MultiHeadAttention_62371515073076.

Math (per batch b, faithful to the reference's quirky softmax over the QUERY axis):
  q/k/v = einsum('nc,chd->nhd', x, W{q,k,v})
  s[i,j,h] = q[i,h,:].k[j,h,:] / 8
  p = softmax over i  (query axis!)
  attnw[i,h] = sum_j p[i,j,h]
             = sum_j exp(s[i,j,h]) / Z[j,h],   Z[j,h] = sum_i exp(s[i,j,h])
  out = einsum('ihd,ohd->io', v * attnw, Wout)

Sharding: batch 8 -> one batch per NeuronCore (data parallel), weights replicated.

Per-core layout strategy (all fp32):
  - Host pre-transposes x -> xt (C,N) and Wout -> wot (HD,O); wq/wk/wv are fed
    natural (C,HD) concatenated, which is already the lhsT layout the PE wants.
  - QKV projections produce transposed Q^T/K^T/V^T [hd, i] tiles directly.
  - Scores are computed transposed, S^T [j, i], per head, K=64 row-packed two
    heads per PE pass (partitions 0-63 / 64-127).
  - exp via ScalarE with fused row-sum (accum_out) -> Z[j]; no max subtraction
    (|s|<~5 so fp32 exp is safe; matches softmax up to fp rounding).
  - attnw computed AND broadcast across each head's 64 d-rows in one step:
    colsum matmul with lhsT = (1/Z) broadcast to 64 columns, two heads
    col-packed (tile_position (0,0)/(0,64)), accumulated over j-tiles in PSUM.
  - applied^T = V^T * attnw_bcast (DVE), then output projection back to
    natural [i, o] layout and DMA out.
"""
import os
import numpy as np
from contextlib import ExitStack

import concourse.bass as bass
import concourse.mybir as mybir
import concourse.tile as tile
from concourse import bacc
from concourse.vector_clock import ScopedClock
from concourse.bass_utils import run_bass_kernel_spmd
import bass_rust

N_CORES = 8
B, N, C, H, D, O = 8, 1024, 256, 8, 64, 256
HD = H * D  # 512
FP32 = mybir.dt.float32
F32R = mybir.dt.float32r
BF16 = mybir.dt.bfloat16
F16 = mybir.dt.float16
EXP = mybir.ActivationFunctionType.Exp


_MAXW = 1  # max sync waits this toolchain's walrus accepts per instruction


class _TC(tile.TileContext):
    """TileContext that splits semaphore waits one-per-instruction.

    The walrus build in this toolchain rejects any instruction carrying more
    than one sync wait ("Too many sync wait commands"), while Tile's
    add_semaphores attaches all needed waits to the consuming instruction.
    Engines execute in order, so moving excess waits onto same-engine NOPs
    emitted immediately before the instruction is semantically identical.
    """

    def _commit_instruction(self, inst, lazy_reg_writes: bool = True):
        si = inst.sync_info
        if (
            si is not None
            and si.on_wait
            and len(si.on_wait) > _MAXW
            and inst.engine != mybir.EngineType.Unassigned
        ):
            waits = list(si.on_wait)
            inst.sync_info = bass_rust.SyncInfo(
                on_wait=waits[-_MAXW:], on_update=list(si.on_update or [])
            )
            for i in range(0, len(waits) - _MAXW, _MAXW):
                nop = self.nc.engines[inst.engine].nop(nofuse=True, hint="waitsplit")
                nop.ins.sync_info = bass_rust.SyncInfo(
                    on_wait=waits[i : i + _MAXW], on_update=[]
                )
        return super()._commit_instruction(inst, lazy_reg_writes)

    def _drain_and_barrier(self, tick_clock, wait_clock):
        probe = self.nc.sync.drain()
        wait_clock.add_sem_waits(
            probe.ins, ScopedClock({None: tick_clock.global_clock})
        )
        si = probe.ins.sync_info
        waits = list(si.on_wait or []) if si is not None else []
        if len(waits) > 1:
            probe.ins.sync_info = bass_rust.SyncInfo(
                on_wait=waits[:1], on_update=list(si.on_update or [])
            )
            for i in range(1, len(waits)):
                d = self.nc.sync.drain()
                d.ins.sync_info = bass_rust.SyncInfo(
                    on_wait=waits[i : i + 1], on_update=[]
                )
        self.nc.all_engine_barrier()
        assert self.sems is not None
        popped = self.nc._tile_sem_poison_stack.pop()
        assert popped is self._sem_poison
        self.nc.clear_and_free_semaphores(list(self.sems.allocated().values()))
        self.nc.all_engine_barrier()


def _bcast64(col_ap):
    """[P,1] AP -> [P,64] AP reading the same element 64x (free step 0)."""
    return bass.AP(col_ap.tensor, col_ap.offset, [list(col_ap.ap[0]), [0, 64]])


def _r(ap):
    """View an fp32 AP as float32r: same bits, full-rate PE matmul."""
    return ap.bitcast(mybir.dt.float32r)


def _emit_body(tc, xt, wqkv, wot, out):
    nc = tc.nc
    with ExitStack() as ctx:
        wpool = ctx.enter_context(tc.tile_pool(name="w", bufs=1))
        qkvpool = ctx.enter_context(tc.tile_pool(name="qkv", bufs=1))
        gpool = ctx.enter_context(tc.tile_pool(name="g", bufs=4))
        stpool = ctx.enter_context(tc.tile_pool(name="st", bufs=4))
        izpool = ctx.enter_context(tc.tile_pool(name="iz", bufs=4))
        zpool = ctx.enter_context(tc.tile_pool(name="z", bufs=2))
        obpool = ctx.enter_context(tc.tile_pool(name="ob", bufs=2))

        # fine-grained input loads, ordered by first use: ic0 halves of x and
        # the q/k weights first so the m=0 projections start ASAP
        XT = [[None, None], [None, None]]   # [kc][ic] -> [128, 512]
        WQC = [[None, None], [None, None], [None, None]]  # [col][kc]
        WOT = []

        def load_x(kc, ic):
            t = wpool.tile([128, 512], F16, tag=f"xt{kc}{ic}", name=f"xt{kc}{ic}")
            nc.sync.dma_start(
                t[:], xt[kc * 128 : (kc + 1) * 128, ic * 512 : (ic + 1) * 512]
            )
            XT[kc][ic] = t

        def load_w(col, kc):
            w = wpool.tile([128, HD], F16, tag=f"w{col}{kc}", name=f"w{col}{kc}")
            nc.sync.dma_start(
                w[:], wqkv[kc * 128 : (kc + 1) * 128, col * HD : (col + 1) * HD]
            )
            WQC[col][kc] = w

        load_x(0, 0); load_x(1, 0); load_w(0, 0); load_w(0, 1)
        load_x(0, 1); load_x(1, 1); load_w(1, 0); load_w(1, 1)
        load_w(2, 0); load_w(2, 1)
        for kt in range(4):
            w = wpool.tile([128, O], F16, tag=f"wot{kt}", name=f"wot{kt}")
            nc.sync.dma_start(w[:], wot[kt * 128 : (kt + 1) * 128, :])
            WOT.append(w)

        QT = [qkvpool.tile([128, N], F16, tag=f"q{m}", name=f"q{m}") for m in range(4)]
        KT = [qkvpool.tile([128, N], F16, tag=f"k{m}", name=f"k{m}") for m in range(4)]
        VT = [qkvpool.tile([128, N], F16, tag=f"v{m}", name=f"v{m}") for m in range(4)]
        APP = [qkvpool.tile([128, N], F16, tag=f"app{m}", name=f"app{m}") for m in range(4)]

        with (
            tc.tile_pool(name="sps", bufs=2, space="PSUM") as sps,
            tc.tile_pool(name="awps", bufs=2, space="PSUM") as awps,
        ):

            def project(col, m, dst):
                """dst[hd', i] = sum_c W[c, col*HD + m*128 + hd'] * xT[c, i]"""
                ps = sps.tile([128, N], FP32, tag="s")
                for ic in range(2):
                    for kc in range(2):
                        nc.tensor.matmul(
                            ps[:, ic * 512 : (ic + 1) * 512],
                            WQC[col][kc][:, m * 128 : (m + 1) * 128],
                            XT[kc][ic][:],
                            start=(kc == 0),
                            stop=(kc == 1),
                        )
                with nc.allow_low_precision(reason="f16 activations"):
                    nc.vector.tensor_copy(dst[:], ps[:])

            for t in range(4):  # head pair (2t, 2t+1)
                project(0, t, QT[t])
                project(1, t, KT[t])
                project(2, t, VT[t])
                aw = awps.tile([128, N], FP32, tag="aw")
                za = zpool.tile([128, 8], FP32, tag="za")
                zb = zpool.tile([128, 8], FP32, tag="zb")
                for jt in range(8):
                    jsl = slice(jt * 128, (jt + 1) * 128)
                    sa = sps.tile([128, N], FP32, tag="s")
                    sb_ = sps.tile([128, N], FP32, tag="s")
                    for ic in range(2):
                        icsl = slice(ic * 512, (ic + 1) * 512)
                        # two K=64 matmuls row-packed in the PE array
                        nc.tensor.matmul(
                            sa[:, icsl], KT[t][0:64, jsl], QT[t][0:64, icsl],
                            start=True, stop=True,
                        )
                        nc.tensor.matmul(
                            sb_[:, icsl], KT[t][64:128, jsl], QT[t][64:128, icsl],
                            start=True, stop=True, tile_position=(64, 0),
                        )
                    # exp: mostly via a DVE psum->sbuf bounce (ScalarE streams
                    # SBUF ~2x faster than PSUM); a slice stays psum-direct to
                    # keep DVE below ACT.
                    ga = gpool.tile([128, N], F16, tag="g")
                    gb = gpool.tile([128, N], F16, tag="g")
                    srcs = []
                    for name_, sps_tile, g_tile, z_tile in (
                        ("a", sa, ga, za), ("b", sb_, gb, zb)
                    ):
                        via_sbuf = name_ == "a" or (jt % 4 != 3)
                        if via_sbuf:
                            st = stpool.tile([128, N], F16, tag="st", name="st")
                            nc.vector.tensor_copy(st[:], sps_tile[:])
                            src = st
                        else:
                            src = sps_tile
                        nc.scalar.activation(
                            g_tile[:], src[:], EXP, scale=0.125,
                            accum_out=z_tile[:, jt : jt + 1],
                        )
                    iza = izpool.tile([128, 64], F16, tag="iz")
                    izb = izpool.tile([128, 64], F16, tag="iz")
                    with nc.allow_low_precision(reason="f32r matmul operands"):
                        nc.vector.reciprocal(iza[:], _bcast64(za[:, jt : jt + 1]))
                        nc.vector.reciprocal(izb[:], _bcast64(zb[:, jt : jt + 1]))
                    for ic in range(2):
                        icsl = slice(ic * 512, (ic + 1) * 512)
                        # attnw (already broadcast over d) accumulated over j,
                        # two heads col-packed
                        nc.tensor.matmul(
                            aw[0:64, icsl], iza[:, 0:64], ga[:, icsl],
                            start=(jt == 0), stop=(jt == 7),
                            tile_position=(0, 0), skip_group_check=True,
                        )
                        nc.tensor.matmul(
                            aw[64:128, icsl], izb[:, 0:64], gb[:, icsl],
                            start=(jt == 0), stop=(jt == 7),
                            tile_position=(0, 64), skip_group_check=True,
                        )
                with nc.allow_low_precision(reason="f16 activations"):
                    nc.vector.tensor_mul(APP[t][:], VT[t][:], aw[:])

        with tc.tile_pool(name="ops", bufs=2, space="PSUM") as ops:
            for it in range(8):
                itsl = slice(it * 128, (it + 1) * 128)
                po = ops.tile([128, O], FP32, tag="o")
                for kt in range(4):
                    nc.tensor.matmul(
                        po[:], APP[kt][:, itsl], WOT[kt][:],
                        start=(kt == 0), stop=(kt == 3),
                    )
                ob = obpool.tile([128, O], FP32, tag="ob")
                nc.vector.tensor_copy(ob[:], po[:])
                nc.sync.dma_start(out[itsl, :], ob[:])


def build_nc(loop=0, use_bacc=False):
    cls = bacc.Bacc if use_bacc else bass.Bass
    nc = cls("TRN2", target_bir_lowering=False, debug=False, num_devices=N_CORES)
    xt = nc.declare_dram_parameter("xt", [C, N], F16, isOutput=False)
    wqkv = nc.declare_dram_parameter("wqkv", [C, 3 * HD], F16, isOutput=False)
    wot = nc.declare_dram_parameter("wot", [HD, O], F16, isOutput=False)
    out = nc.declare_dram_parameter("out", [N, O], FP32, isOutput=True)
    with _TC(nc, num_cores=N_CORES) as tc:
        if loop:
            with tc.For_i(0, loop, 1):
                _emit_body(tc, xt.ap(), wqkv.ap(), wot.ap(), out.ap())
        else:
            _emit_body(tc, xt.ap(), wqkv.ap(), wot.ap(), out.ap())
    return nc


def make_in_maps(features, weight_q, weight_k, weight_v, weight_out):
    wqkv = np.ascontiguousarray(
        np.concatenate(
            [
                weight_q.reshape(C, HD),
                weight_k.reshape(C, HD),
                weight_v.reshape(C, HD),
            ],
            axis=1,
        ),
        dtype=np.float16,
    )
    wot = np.ascontiguousarray(weight_out.reshape(O, HD).T, dtype=np.float16)
    in_maps = []
    for b in range(B):
        xt = np.ascontiguousarray(features[b].T, dtype=np.float16)
        in_maps.append({"xt": xt, "wqkv": wqkv, "wot": wot})
    return in_maps


_CACHED_NC = None


def kernel(features, weight_q, weight_k, weight_v, weight_out):
    global _CACHED_NC
    if _CACHED_NC is None:
        _CACHED_NC = build_nc(loop=0)
    in_maps = make_in_maps(
        np.asarray(features, np.float32),
        np.asarray(weight_q, np.float32),
        np.asarray(weight_k, np.float32),
        np.asarray(weight_v, np.float32),
        np.asarray(weight_out, np.float32),
    )
    res = run_bass_kernel_spmd(_CACHED_NC, in_maps, list(range(N_CORES)))
    return np.stack([res.results[b]["out"] for b in range(B)], axis=0)


if __name__ == "__main__":
    rng = np.random.default_rng(0)
    feats = rng.standard_normal((B, N, C)).astype(np.float32)
    wq = rng.standard_normal((C, H, D)).astype(np.float32) * 0.05
    wk = rng.standard_normal((C, H, D)).astype(np.float32) * 0.05
    wv = rng.standard_normal((C, H, D)).astype(np.float32) * 0.05
    wo = rng.standard_normal((O, H, D)).astype(np.float32) * 0.05
    o = kernel(feats, wq, wk, wv, wo)
    print("kernel ran, out shape", o.shape, "finite:", np.isfinite(o).all())



# revision 2
# speedup vs baseline: 1.2896x; 1.2896x over previous
"""Trainium2 Bass kernel for nn_MultiHeadAttention_62371515073076.

Math (per batch b, faithful to the reference's quirky softmax over the QUERY
axis):
  q/k/v = einsum('nc,chd->nhd', x, W{q,k,v})
  s[i,j,h] = q[i,h,:].k[j,h,:] / 8
  p = softmax over i  (query axis!)
  attnw[i,h] = sum_j p[i,j,h] = sum_j exp(s[i,j,h]) / Z[j,h],
               Z[j,h] = sum_i exp(s[i,j,h])
  out = einsum('ihd,ohd->io', v * attnw, Wout)

Sharding: batch 8 -> one batch per NeuronCore (data parallel), weights
replicated. Host pre-transposes x -> xt (C,N) and Wout -> wot (HD,O); the
q/k/v weights are fed natural (C,HD) concatenated (already the lhsT layout).

Per-core dataflow (engine-balanced against the measured TRN2 rates:
ACT 1 cyc/elem from SBUF but ~2 from PSUM; DVE ~1 cyc/elem any source;
GPSIMD and DMA cannot access PSUM at all, so every PSUM byte is drained by
DVE or ACT):

  - QKV projections -> Q^T/K^T/V^T [hd, i] f16 tiles (PE, K=128 matmuls;
    PSUM drained by DVE). Projections for head-pair t+1 are interleaved
    into t's j-loop so ACT never idles at pair boundaries.
  - Scores S^T [j, i] per head, two heads row-packed in the PE array
    (tile_position (0,0)/(64,0)).
  - exp via ACT with fused row-sum (accum_out) -> Z columns; most tiles are
    first bounced PSUM->SBUF f16 by DVE (ACT streams SBUF 2x faster than
    PSUM); a measured subset reads PSUM directly to use ACT's slack
    (DIRECT_JTS). No max-subtraction: |s| <~ 5 so fp32/f16 exp is safe.
  - The j-loop is software-pipelined: scores(jt) are emitted before
    colsum(jt-1) and recip(jt-1) after bounces(jt), so neither PE nor DVE
    ever stalls on ACT in program order.
  - attnw accumulated over j by PE: lhsT = (1/Z) broadcast via a stride-0
    AP to 64 columns, two heads col-packed ((0,0)/(0,64)), PSUM-accumulated
    across the 8 j-tiles.
  - applied^T = V^T * attnw_bcast (DVE), then the output projection runs in
    two 4-chunk waves through the attnw PSUM slot (no extra PSUM pool; PSUM
    is exactly 8 banks: 3 x score-slot (2) + attnw/out-wave slot (2)).
  - All tile pools are created once and shared across timing-loop bodies
    (weights/qkv double-buffered) so consecutive bodies overlap; timing
    builds unroll 4 bodies per For_i trip to amortize Tile's per-trip
    drain/barrier.
"""
import numpy as np
from contextlib import ExitStack

import concourse.bass as bass
import concourse.mybir as mybir
import concourse.tile as tile
from concourse import bacc
from concourse.vector_clock import ScopedClock
from concourse.bass_utils import run_bass_kernel_spmd
import bass_rust

N_CORES = 8
B, N, C, H, D, O = 8, 1024, 256, 8, 64, 256
HD = H * D  # 512
FP32 = mybir.dt.float32
F16 = mybir.dt.float16
EXP = mybir.ActivationFunctionType.Exp

# Per-head-pair jt indices whose b-half exp reads PSUM directly on ACT
# (balances the DVE bottleneck against ACT's slack).
DIRECT_JTS = [(1, 3, 5, 7), (), (1, 3, 5, 7), ()]

_MAXW = 1  # max sync waits this toolchain's walrus accepts per instruction


class _TC(tile.TileContext):
    """TileContext that splits semaphore waits one-per-instruction.

    The walrus build in this toolchain rejects any instruction carrying more
    than one sync wait ("Too many sync wait commands"), while Tile's
    add_semaphores attaches all needed waits to the consuming instruction.
    Engines execute in order, so moving excess waits onto same-engine NOPs
    emitted immediately before the instruction is semantically identical.
    """

    def _commit_instruction(self, inst, lazy_reg_writes: bool = True):
        si = inst.sync_info
        if (
            si is not None
            and si.on_wait
            and len(si.on_wait) > _MAXW
            and inst.engine != mybir.EngineType.Unassigned
        ):
            waits = list(si.on_wait)
            inst.sync_info = bass_rust.SyncInfo(
                on_wait=waits[-_MAXW:], on_update=list(si.on_update or [])
            )
            for i in range(0, len(waits) - _MAXW, _MAXW):
                nop = self.nc.engines[inst.engine].nop(nofuse=True, hint="waitsplit")
                nop.ins.sync_info = bass_rust.SyncInfo(
                    on_wait=waits[i : i + _MAXW], on_update=[]
                )
        return super()._commit_instruction(inst, lazy_reg_writes)

    def _drain_and_barrier(self, tick_clock, wait_clock):
        probe = self.nc.sync.drain()
        wait_clock.add_sem_waits(
            probe.ins, ScopedClock({None: tick_clock.global_clock})
        )
        si = probe.ins.sync_info
        waits = list(si.on_wait or []) if si is not None else []
        if len(waits) > 1:
            probe.ins.sync_info = bass_rust.SyncInfo(
                on_wait=waits[:1], on_update=list(si.on_update or [])
            )
            for i in range(1, len(waits)):
                d = self.nc.sync.drain()
                d.ins.sync_info = bass_rust.SyncInfo(
                    on_wait=waits[i : i + 1], on_update=[]
                )
        self.nc.all_engine_barrier()
        assert self.sems is not None
        popped = self.nc._tile_sem_poison_stack.pop()
        assert popped is self._sem_poison
        self.nc.clear_and_free_semaphores(list(self.sems.allocated().values()))
        self.nc.all_engine_barrier()


def _bcast64(col_ap):
    """[P,1] AP -> [P,64] AP reading the same element 64x (free step 0)."""
    return bass.AP(col_ap.tensor, col_ap.offset, [list(col_ap.ap[0]), [0, 64]])


def _r(ap):
    """View an fp32 AP as float32r: same bits, full-rate PE matmul."""
    return ap.bitcast(mybir.dt.float32r)


def _emit_body(tc, pools, xt, wqkv, wot, out):
    nc = tc.nc
    wpool, qkvpool, gpool, stpool, rzpool, zpool, obpool, sps, awps = pools

    XT = [[None, None], [None, None]]
    WQC = [[None, None], [None, None], [None, None]]
    WOT = []

    def load_x(kc, ic):
        t = wpool.tile([128, 512], F16, tag=f"xt{kc}{ic}", name=f"xt{kc}{ic}")
        nc.sync.dma_start(
            t[:], xt[kc * 128 : (kc + 1) * 128, ic * 512 : (ic + 1) * 512]
        )
        XT[kc][ic] = t

    def load_w(col, kc):
        w = wpool.tile([128, HD], F16, tag=f"w{col}{kc}", name=f"w{col}{kc}")
        nc.sync.dma_start(
            w[:], wqkv[kc * 128 : (kc + 1) * 128, col * HD : (col + 1) * HD]
        )
        WQC[col][kc] = w

    load_x(0, 0); load_x(1, 0); load_w(0, 0); load_w(0, 1)
    load_x(0, 1); load_x(1, 1); load_w(1, 0); load_w(1, 1)
    load_w(2, 0); load_w(2, 1)
    for kt in range(4):
        w = wpool.tile([128, O], F16, tag=f"wot{kt}", name=f"wot{kt}")
        nc.sync.dma_start(w[:], wot[kt * 128 : (kt + 1) * 128, :])
        WOT.append(w)

    QT = [qkvpool.tile([128, N], F16, tag=f"q{m}", name=f"q{m}") for m in range(4)]
    KT = [qkvpool.tile([128, N], F16, tag=f"k{m}", name=f"k{m}") for m in range(4)]
    VT = [qkvpool.tile([128, N], F16, tag=f"v{m}", name=f"v{m}") for m in range(4)]
    APP = [qkvpool.tile([128, N], F16, tag=f"app{m}", name=f"app{m}") for m in range(4)]

    def project(col, m, dst):
        ps = sps.tile([128, N], FP32, tag="s")
        for ic in range(2):
            for kc in range(2):
                nc.tensor.matmul(
                    ps[:, ic * 512 : (ic + 1) * 512],
                    WQC[col][kc][:, m * 128 : (m + 1) * 128],
                    XT[kc][ic][:],
                    start=(kc == 0),
                    stop=(kc == 1),
                )
        with nc.allow_low_precision(reason="f16 activations"):
            nc.vector.tensor_copy(dst[:], ps[:])

    QKV = (QT, KT, VT)
    project(0, 0, QT[0])
    project(1, 0, KT[0])
    project(2, 0, VT[0])
    for t in range(4):
        aw = awps.tile([128, N], FP32, tag="aw")
        z = zpool.tile([128, 16], FP32, tag="z")
        pending = None  # previous jt whose recip+colsum are not yet emitted

        def colsum(ga, gb, rz, jt):
            for ic in range(2):
                icsl = slice(ic * 512, (ic + 1) * 512)
                nc.tensor.matmul(
                    aw[0:64, icsl], _bcast64(rz[:, 0:1]), ga[:, icsl],
                    start=(jt == 0), stop=(jt == 7),
                    tile_position=(0, 0), skip_group_check=True,
                )
                nc.tensor.matmul(
                    aw[64:128, icsl], _bcast64(rz[:, 1:2]), gb[:, icsl],
                    start=(jt == 0), stop=(jt == 7),
                    tile_position=(0, 64), skip_group_check=True,
                )

        def recip(jt):
            rz = rzpool.tile([128, 2], F16, tag="rz")
            with nc.allow_low_precision(reason="f16 matmul lhsT"):
                nc.vector.reciprocal(rz[:], z[:, 2 * jt : 2 * jt + 2])
            return rz

        for jt in range(8):
            jsl = slice(jt * 128, (jt + 1) * 128)
            sa = sps.tile([128, N], FP32, tag="s")
            sb_ = sps.tile([128, N], FP32, tag="s")
            for ic in range(2):
                icsl = slice(ic * 512, (ic + 1) * 512)
                nc.tensor.matmul(
                    sa[:, icsl], KT[t][0:64, jsl], QT[t][0:64, icsl],
                    start=True, stop=True,
                )
                nc.tensor.matmul(
                    sb_[:, icsl], KT[t][64:128, jsl], QT[t][64:128, icsl],
                    start=True, stop=True, tile_position=(64, 0),
                )
            ga = gpool.tile([128, N], F16, tag="g")
            gb = gpool.tile([128, N], F16, tag="g")
            for half, sps_tile, g_tile in ((0, sa, ga), (1, sb_, gb)):
                direct = half == 1 and jt in DIRECT_JTS[t]
                if direct:
                    src = sps_tile
                else:
                    st = stpool.tile([128, N], F16, tag="st", name="st")
                    nc.vector.tensor_copy(st[:], sps_tile[:])
                    src = st
                nc.scalar.activation(
                    g_tile[:], src[:], EXP, scale=0.125,
                    accum_out=z[:, 2 * jt + half : 2 * jt + half + 1],
                )
            if pending is not None:
                pga, pgb, pjt = pending
                colsum(pga, pgb, recip(pjt), pjt)
            pending = (ga, gb, jt)
            if t < 3 and jt >= 5:
                col = jt - 5
                project(col, t + 1, QKV[col][t + 1])
        pga, pgb, pjt = pending
        colsum(pga, pgb, recip(pjt), pjt)
        with nc.allow_low_precision(reason="f16 activations"):
            nc.vector.tensor_mul(APP[t][:], VT[t][:], aw[:])

    # Output projection in two 4-chunk waves through the aw PSUM slot (same
    # tag+shape as aw, so no extra PSUM pool; naturally ordered after the
    # last APP-mul has consumed aw).
    for w in range(2):
        po = awps.tile([128, N], FP32, tag="aw")
        for sub in range(4):
            it = w * 4 + sub
            itsl = slice(it * 128, (it + 1) * 128)
            for kt in range(4):
                nc.tensor.matmul(
                    po[:, sub * O : (sub + 1) * O], APP[kt][:, itsl], WOT[kt][:],
                    start=(kt == 0), stop=(kt == 3),
                )
        ob = obpool.tile([128, N], FP32, tag="ob")
        nc.vector.tensor_copy(ob[:], po[:])
        # ob partition p, flat free (it 4, col 256) -> out rows w*512+it*128+p
        dst = bass.AP(
            out.tensor,
            out.offset + (w * 512) * O,
            [[O, 128], [128 * O, 4], [1, O]],
        )
        nc.sync.dma_start(dst, ob[:])


def build_nc(loop=0, unroll=4, use_bacc=False):
    """loop=0: single body (the graded kernel). loop=L: L body executions
    for timing, emitted as For_i(L // unroll) trips x unroll bodies (plus a
    straight-line remainder) to amortize Tile's per-trip drain/barrier."""
    cls = bacc.Bacc if use_bacc else bass.Bass
    nc = cls("TRN2", target_bir_lowering=False, debug=False, num_devices=N_CORES)
    xt = nc.declare_dram_parameter("xt", [C, N], F16, isOutput=False)
    wqkv = nc.declare_dram_parameter("wqkv", [C, 3 * HD], F16, isOutput=False)
    wot = nc.declare_dram_parameter("wot", [HD, O], F16, isOutput=False)
    out = nc.declare_dram_parameter("out", [N, O], FP32, isOutput=True)
    with _TC(nc, num_cores=N_CORES) as tc:
        with ExitStack() as ctx:
            pools = (
                ctx.enter_context(tc.tile_pool(name="w", bufs=2)),
                ctx.enter_context(tc.tile_pool(name="qkv", bufs=2)),
                ctx.enter_context(tc.tile_pool(name="g", bufs=4)),
                ctx.enter_context(tc.tile_pool(name="st", bufs=4)),
                ctx.enter_context(tc.tile_pool(name="rz", bufs=4)),
                ctx.enter_context(tc.tile_pool(name="z", bufs=2)),
                ctx.enter_context(tc.tile_pool(name="ob", bufs=2)),
                ctx.enter_context(tc.tile_pool(name="sps", bufs=3, space="PSUM")),
                ctx.enter_context(tc.tile_pool(name="awps", bufs=1, space="PSUM")),
            )
            args = (tc, pools, xt.ap(), wqkv.ap(), wot.ap(), out.ap())
            trips, rem = divmod(loop, unroll)
            if loop == 0:
                _emit_body(*args)
            else:
                if trips:
                    with tc.For_i(0, trips, 1):
                        for _ in range(unroll):
                            _emit_body(*args)
                for _ in range(rem):
                    _emit_body(*args)
    return nc


def make_in_maps(features, weight_q, weight_k, weight_v, weight_out):
    wqkv = np.ascontiguousarray(
        np.concatenate(
            [
                weight_q.reshape(C, HD),
                weight_k.reshape(C, HD),
                weight_v.reshape(C, HD),
            ],
            axis=1,
        ),
        dtype=np.float16,
    )
    wot = np.ascontiguousarray(weight_out.reshape(O, HD).T, dtype=np.float16)
    in_maps = []
    for b in range(B):
        xt = np.ascontiguousarray(features[b].T, dtype=np.float16)
        in_maps.append({"xt": xt, "wqkv": wqkv, "wot": wot})
    return in_maps


_CACHED_NC = None


def kernel(features, weight_q, weight_k, weight_v, weight_out):
    global _CACHED_NC
    if _CACHED_NC is None:
        _CACHED_NC = build_nc(loop=0)
    in_maps = make_in_maps(
        np.asarray(features, np.float32),
        np.asarray(weight_q, np.float32),
        np.asarray(weight_k, np.float32),
        np.asarray(weight_v, np.float32),
        np.asarray(weight_out, np.float32),
    )
    res = run_bass_kernel_spmd(_CACHED_NC, in_maps, list(range(N_CORES)))
    return np.stack([res.results[b]["out"] for b in range(B)], axis=0)


if __name__ == "__main__":
    rng = np.random.default_rng(0)
    feats = rng.standard_normal((B, N, C)).astype(np.float32)
    wq = rng.standard_normal((C, H, D)).astype(np.float32) * 0.05
    wk = rng.standard_normal((C, H, D)).astype(np.float32) * 0.05
    wv = rng.standard_normal((C, H, D)).astype(np.float32) * 0.05
    wo = rng.standard_normal((O, H, D)).astype(np.float32) * 0.05
    o = kernel(feats, wq, wk, wv, wo)
    print("kernel ran, out shape", o.shape, "finite:", np.isfinite(o).all())


# revision 3
# speedup vs baseline: 1.2899x; 1.0003x over previous
"""Trainium2 Bass kernel for nn_MultiHeadAttention_62371515073076.

Math (per batch b, faithful to the reference's quirky softmax over the QUERY
axis):
  q/k/v = einsum('nc,chd->nhd', x, W{q,k,v})
  s[i,j,h] = q[i,h,:].k[j,h,:] / 8
  p = softmax over i  (query axis!)
  attnw[i,h] = sum_j p[i,j,h] = sum_j exp(s[i,j,h]) / Z[j,h],
               Z[j,h] = sum_i exp(s[i,j,h])
  out = einsum('ihd,ohd->io', v * attnw, Wout)

Sharding: batch 8 -> one batch per NeuronCore (data parallel), weights
replicated. Host pre-transposes x -> xt (C,N) and Wout -> wot (HD,O); the
q/k/v weights are fed natural (C,HD) concatenated (already the lhsT layout).

Per-core dataflow (engine-balanced against the measured TRN2 rates:
ACT 1 cyc/elem from SBUF but ~2 from PSUM; DVE ~1 cyc/elem any source;
GPSIMD and DMA cannot access PSUM at all, so every PSUM byte is drained by
DVE or ACT):

  - QKV projections -> Q^T/K^T/V^T [hd, i] f16 tiles (PE, K=128 matmuls;
    PSUM drained by DVE). Projections for head-pair t+1 are interleaved
    into t's j-loop so ACT never idles at pair boundaries.
  - Scores S^T [j, i] per head, two heads row-packed in the PE array
    (tile_position (0,0)/(64,0)).
  - exp via ACT with fused row-sum (accum_out) -> Z columns; most tiles are
    first bounced PSUM->SBUF f16 by DVE (ACT streams SBUF 2x faster than
    PSUM); a measured subset reads PSUM directly to use ACT's slack
    (DIRECT_JTS). No max-subtraction: |s| <~ 5 so fp32/f16 exp is safe.
  - The j-loop is software-pipelined: scores(jt) are emitted before
    colsum(jt-1) and recip(jt-1) after bounces(jt), so neither PE nor DVE
    ever stalls on ACT in program order.
  - attnw accumulated over j by PE: lhsT = (1/Z) broadcast via a stride-0
    AP to 64 columns, two heads col-packed ((0,0)/(0,64)), PSUM-accumulated
    across the 8 j-tiles.
  - applied^T = V^T * attnw_bcast (DVE), then the output projection runs in
    two 4-chunk waves through the attnw PSUM slot (no extra PSUM pool; PSUM
    is exactly 8 banks: 3 x score-slot (2) + attnw/out-wave slot (2)).
  - All tile pools are created once and shared across timing-loop bodies
    (weights/qkv double-buffered) so consecutive bodies overlap; timing
    builds unroll 4 bodies per For_i trip to amortize Tile's per-trip
    drain/barrier.
"""
import numpy as np
from contextlib import ExitStack

import concourse.bass as bass
import concourse.mybir as mybir
import concourse.tile as tile
from concourse import bacc
from concourse.vector_clock import ScopedClock
from concourse.bass_utils import run_bass_kernel_spmd
import bass_rust

N_CORES = 8
B, N, C, H, D, O = 8, 1024, 256, 8, 64, 256
HD = H * D  # 512
FP32 = mybir.dt.float32
F16 = mybir.dt.float16
EXP = mybir.ActivationFunctionType.Exp

# Per-head-pair jt indices whose b-half exp reads PSUM directly on ACT
# (balances the DVE bottleneck against ACT's slack).
DIRECT_JTS = [(1, 3, 5, 7), (), (1, 3, 5, 7), ()]

_MAXW = 1  # max sync waits this toolchain's walrus accepts per instruction


class _TC(tile.TileContext):
    """TileContext that splits semaphore waits one-per-instruction.

    The walrus build in this toolchain rejects any instruction carrying more
    than one sync wait ("Too many sync wait commands"), while Tile's
    add_semaphores attaches all needed waits to the consuming instruction.
    Engines execute in order, so moving excess waits onto same-engine NOPs
    emitted immediately before the instruction is semantically identical.
    """

    def _commit_instruction(self, inst, lazy_reg_writes: bool = True):
        si = inst.sync_info
        if (
            si is not None
            and si.on_wait
            and len(si.on_wait) > _MAXW
            and inst.engine != mybir.EngineType.Unassigned
        ):
            waits = list(si.on_wait)
            inst.sync_info = bass_rust.SyncInfo(
                on_wait=waits[-_MAXW:], on_update=list(si.on_update or [])
            )
            for i in range(0, len(waits) - _MAXW, _MAXW):
                nop = self.nc.engines[inst.engine].nop(nofuse=True, hint="waitsplit")
                nop.ins.sync_info = bass_rust.SyncInfo(
                    on_wait=waits[i : i + _MAXW], on_update=[]
                )
        return super()._commit_instruction(inst, lazy_reg_writes)

    def _drain_and_barrier(self, tick_clock, wait_clock):
        probe = self.nc.sync.drain()
        wait_clock.add_sem_waits(
            probe.ins, ScopedClock({None: tick_clock.global_clock})
        )
        si = probe.ins.sync_info
        waits = list(si.on_wait or []) if si is not None else []
        if len(waits) > 1:
            probe.ins.sync_info = bass_rust.SyncInfo(
                on_wait=waits[:1], on_update=list(si.on_update or [])
            )
            for i in range(1, len(waits)):
                d = self.nc.sync.drain()
                d.ins.sync_info = bass_rust.SyncInfo(
                    on_wait=waits[i : i + 1], on_update=[]
                )
        self.nc.all_engine_barrier()
        assert self.sems is not None
        popped = self.nc._tile_sem_poison_stack.pop()
        assert popped is self._sem_poison
        self.nc.clear_and_free_semaphores(list(self.sems.allocated().values()))
        self.nc.all_engine_barrier()


def _bcast64(col_ap):
    """[P,1] AP -> [P,64] AP reading the same element 64x (free step 0)."""
    return bass.AP(col_ap.tensor, col_ap.offset, [list(col_ap.ap[0]), [0, 64]])


def _r(ap):
    """View an fp32 AP as float32r: same bits, full-rate PE matmul."""
    return ap.bitcast(mybir.dt.float32r)


def _emit_body(tc, pools, xt, wqkv, wot, out):
    nc = tc.nc
    wpool, qkvpool, gpool, stpool, rzpool, zpool, obpool, sps, awps = pools

    XT = [[None, None], [None, None]]
    WQC = [[None, None], [None, None], [None, None]]
    WOT = []

    def load_x(kc, ic):
        t = wpool.tile([128, 512], F16, tag=f"xt{kc}{ic}", name=f"xt{kc}{ic}")
        nc.sync.dma_start(
            t[:], xt[kc * 128 : (kc + 1) * 128, ic * 512 : (ic + 1) * 512]
        )
        XT[kc][ic] = t

    def load_w(col, kc):
        w = wpool.tile([128, HD], F16, tag=f"w{col}{kc}", name=f"w{col}{kc}")
        nc.sync.dma_start(
            w[:], wqkv[kc * 128 : (kc + 1) * 128, col * HD : (col + 1) * HD]
        )
        WQC[col][kc] = w

    load_x(0, 0); load_x(1, 0); load_w(0, 0); load_w(0, 1)
    load_x(0, 1); load_x(1, 1); load_w(1, 0); load_w(1, 1)
    load_w(2, 0); load_w(2, 1)
    for kt in range(4):
        w = wpool.tile([128, O], F16, tag=f"wot{kt}", name=f"wot{kt}")
        nc.sync.dma_start(w[:], wot[kt * 128 : (kt + 1) * 128, :])
        WOT.append(w)

    QT = [qkvpool.tile([128, N], F16, tag=f"q{m}", name=f"q{m}") for m in range(4)]
    KT = [qkvpool.tile([128, N], F16, tag=f"k{m}", name=f"k{m}") for m in range(4)]
    VT = [qkvpool.tile([128, N], F16, tag=f"v{m}", name=f"v{m}") for m in range(4)]
    APP = [qkvpool.tile([128, N], F16, tag=f"app{m}", name=f"app{m}") for m in range(4)]

    def project(col, m, dst):
        ps = sps.tile([128, N], FP32, tag="s")
        for ic in range(2):
            for kc in range(2):
                nc.tensor.matmul(
                    ps[:, ic * 512 : (ic + 1) * 512],
                    WQC[col][kc][:, m * 128 : (m + 1) * 128],
                    XT[kc][ic][:],
                    start=(kc == 0),
                    stop=(kc == 1),
                )
        with nc.allow_low_precision(reason="f16 activations"):
            nc.vector.tensor_copy(dst[:], ps[:])

    QKV = (QT, KT, VT)
    project(0, 0, QT[0])
    project(1, 0, KT[0])
    project(2, 0, VT[0])
    for t in range(4):
        aw = awps.tile([128, N], FP32, tag="aw")
        z = zpool.tile([128, 16], FP32, tag="z")
        pending = None  # previous jt whose recip+colsum are not yet emitted

        def colsum(ga, gb, rz, jt):
            for ic in range(2):
                icsl = slice(ic * 512, (ic + 1) * 512)
                nc.tensor.matmul(
                    aw[0:64, icsl], _bcast64(rz[:, 0:1]), ga[:, icsl],
                    start=(jt == 0), stop=(jt == 7),
                    tile_position=(0, 0), skip_group_check=True,
                )
                nc.tensor.matmul(
                    aw[64:128, icsl], _bcast64(rz[:, 1:2]), gb[:, icsl],
                    start=(jt == 0), stop=(jt == 7),
                    tile_position=(0, 64), skip_group_check=True,
                )

        def recip(jt):
            rz = rzpool.tile([128, 2], F16, tag="rz")
            with nc.allow_low_precision(reason="f16 matmul lhsT"):
                nc.vector.reciprocal(rz[:], z[:, 2 * jt : 2 * jt + 2])
            return rz

        for jt in range(8):
            jsl = slice(jt * 128, (jt + 1) * 128)
            sa = sps.tile([128, N], FP32, tag="s")
            sb_ = sps.tile([128, N], FP32, tag="s")
            # sa's two chunks first so its DVE bounce starts one matmul sooner
            for ic in range(2):
                icsl = slice(ic * 512, (ic + 1) * 512)
                nc.tensor.matmul(
                    sa[:, icsl], KT[t][0:64, jsl], QT[t][0:64, icsl],
                    start=True, stop=True,
                )
            for ic in range(2):
                icsl = slice(ic * 512, (ic + 1) * 512)
                nc.tensor.matmul(
                    sb_[:, icsl], KT[t][64:128, jsl], QT[t][64:128, icsl],
                    start=True, stop=True, tile_position=(64, 0),
                )
            ga = gpool.tile([128, N], F16, tag="g")
            gb = gpool.tile([128, N], F16, tag="g")
            for half, sps_tile, g_tile in ((0, sa, ga), (1, sb_, gb)):
                direct = half == 1 and jt in DIRECT_JTS[t]
                if direct:
                    src = sps_tile
                else:
                    st = stpool.tile([128, N], F16, tag="st", name="st")
                    nc.vector.tensor_copy(st[:], sps_tile[:])
                    src = st
                nc.scalar.activation(
                    g_tile[:], src[:], EXP, scale=0.125,
                    accum_out=z[:, 2 * jt + half : 2 * jt + half + 1],
                )
            if pending is not None:
                pga, pgb, pjt = pending
                colsum(pga, pgb, recip(pjt), pjt)
            pending = (ga, gb, jt)
            if t < 3 and jt >= 5:
                col = jt - 5
                project(col, t + 1, QKV[col][t + 1])
        pga, pgb, pjt = pending
        colsum(pga, pgb, recip(pjt), pjt)
        with nc.allow_low_precision(reason="f16 activations"):
            nc.vector.tensor_mul(APP[t][:], VT[t][:], aw[:])

    # Output projection in two 4-chunk waves through the aw PSUM slot (same
    # tag+shape as aw, so no extra PSUM pool; naturally ordered after the
    # last APP-mul has consumed aw).
    for w in range(2):
        po = awps.tile([128, N], FP32, tag="aw")
        for sub in range(4):
            it = w * 4 + sub
            itsl = slice(it * 128, (it + 1) * 128)
            for kt in range(4):
                nc.tensor.matmul(
                    po[:, sub * O : (sub + 1) * O], APP[kt][:, itsl], WOT[kt][:],
                    start=(kt == 0), stop=(kt == 3),
                )
        ob = obpool.tile([128, N], FP32, tag="ob")
        nc.vector.tensor_copy(ob[:], po[:])
        # ob partition p, flat free (it 4, col 256) -> out rows w*512+it*128+p
        dst = bass.AP(
            out.tensor,
            out.offset + (w * 512) * O,
            [[O, 128], [128 * O, 4], [1, O]],
        )
        nc.sync.dma_start(dst, ob[:])


def build_nc(loop=0, unroll=4, use_bacc=False):
    """loop=0: single body (the graded kernel). loop=L: L body executions
    for timing, emitted as For_i(L // unroll) trips x unroll bodies (plus a
    straight-line remainder) to amortize Tile's per-trip drain/barrier."""
    cls = bacc.Bacc if use_bacc else bass.Bass
    nc = cls("TRN2", target_bir_lowering=False, debug=False, num_devices=N_CORES)
    xt = nc.declare_dram_parameter("xt", [C, N], F16, isOutput=False)
    wqkv = nc.declare_dram_parameter("wqkv", [C, 3 * HD], F16, isOutput=False)
    wot = nc.declare_dram_parameter("wot", [HD, O], F16, isOutput=False)
    out = nc.declare_dram_parameter("out", [N, O], FP32, isOutput=True)
    with _TC(nc, num_cores=N_CORES) as tc:
        with ExitStack() as ctx:
            pools = (
                ctx.enter_context(tc.tile_pool(name="w", bufs=2)),
                ctx.enter_context(tc.tile_pool(name="qkv", bufs=2)),
                ctx.enter_context(tc.tile_pool(name="g", bufs=4)),
                ctx.enter_context(tc.tile_pool(name="st", bufs=4)),
                ctx.enter_context(tc.tile_pool(name="rz", bufs=4)),
                ctx.enter_context(tc.tile_pool(name="z", bufs=2)),
                ctx.enter_context(tc.tile_pool(name="ob", bufs=2)),
                ctx.enter_context(tc.tile_pool(name="sps", bufs=3, space="PSUM")),
                ctx.enter_context(tc.tile_pool(name="awps", bufs=1, space="PSUM")),
            )
            args = (tc, pools, xt.ap(), wqkv.ap(), wot.ap(), out.ap())
            trips, rem = divmod(loop, unroll)
            if loop == 0:
                _emit_body(*args)
            else:
                if trips:
                    with tc.For_i(0, trips, 1):
                        for _ in range(unroll):
                            _emit_body(*args)
                for _ in range(rem):
                    _emit_body(*args)
    return nc


def make_in_maps(features, weight_q, weight_k, weight_v, weight_out):
    wqkv = np.ascontiguousarray(
        np.concatenate(
            [
                weight_q.reshape(C, HD),
                weight_k.reshape(C, HD),
                weight_v.reshape(C, HD),
            ],
            axis=1,
        ),
        dtype=np.float16,
    )
    wot = np.ascontiguousarray(weight_out.reshape(O, HD).T, dtype=np.float16)
    in_maps = []
    for b in range(B):
        xt = np.ascontiguousarray(features[b].T, dtype=np.float16)
        in_maps.append({"xt": xt, "wqkv": wqkv, "wot": wot})
    return in_maps


_CACHED_NC = None


def kernel(features, weight_q, weight_k, weight_v, weight_out):
    global _CACHED_NC
    if _CACHED_NC is None:
        _CACHED_NC = build_nc(loop=0)
    in_maps = make_in_maps(
        np.asarray(features, np.float32),
        np.asarray(weight_q, np.float32),
        np.asarray(weight_k, np.float32),
        np.asarray(weight_v, np.float32),
        np.asarray(weight_out, np.float32),
    )
    res = run_bass_kernel_spmd(_CACHED_NC, in_maps, list(range(N_CORES)))
    return np.stack([res.results[b]["out"] for b in range(B)], axis=0)


if __name__ == "__main__":
    rng = np.random.default_rng(0)
    feats = rng.standard_normal((B, N, C)).astype(np.float32)
    wq = rng.standard_normal((C, H, D)).astype(np.float32) * 0.05
    wk = rng.standard_normal((C, H, D)).astype(np.float32) * 0.05
    wv = rng.standard_normal((C, H, D)).astype(np.float32) * 0.05
    wo = rng.standard_normal((O, H, D)).astype(np.float32) * 0.05
    o = kernel(feats, wq, wk, wv, wo)
    print("kernel ran, out shape", o.shape, "finite:", np.isfinite(o).all())


# revision 7
# speedup vs baseline: 1.2953x; 1.0041x over previous
"""Trainium2 Bass kernel for nn_MultiHeadAttention_62371515073076.

Math (per batch b, faithful to the reference's quirky softmax over the QUERY
axis):
  q/k/v = einsum('nc,chd->nhd', x, W{q,k,v})
  s[i,j,h] = q[i,h,:].k[j,h,:] / 8
  p = softmax over i  (query axis!)
  attnw[i,h] = sum_j p[i,j,h] = sum_j exp(s[i,j,h]) / Z[j,h],
               Z[j,h] = sum_i exp(s[i,j,h])
  out = einsum('ihd,ohd->io', v * attnw, Wout)

Sharding: batch 8 -> one batch per NeuronCore (data parallel), weights
replicated. Host pre-transposes x -> xt (C,N) and Wout -> wot (HD,O); the
q/k/v weights are fed natural (C,HD) concatenated (already the lhsT layout).

Per-core dataflow (engine-balanced against the measured TRN2 rates:
ACT 1 cyc/elem from SBUF but ~2 from PSUM; DVE ~1 cyc/elem any source;
GPSIMD and DMA cannot access PSUM at all, so every PSUM byte is drained by
DVE or ACT):

  - QKV projections -> Q^T/K^T/V^T [hd, i] f16 tiles (PE, K=128 matmuls;
    PSUM drained by DVE). Projections for head-pair t+1 are interleaved
    into t's j-loop so ACT never idles at pair boundaries.
  - Scores S^T [j, i] per head, two heads row-packed in the PE array
    (tile_position (0,0)/(64,0)).
  - exp via ACT with fused row-sum (accum_out) -> Z columns; most tiles are
    first bounced PSUM->SBUF f16 by DVE (ACT streams SBUF 2x faster than
    PSUM); a measured subset reads PSUM directly to use ACT's slack
    (DIRECT_JTS). No max-subtraction: |s| <~ 5 so fp32/f16 exp is safe.
  - The j-loop is software-pipelined: scores(jt) are emitted before
    colsum(jt-1) and recip(jt-1) after bounces(jt), so neither PE nor DVE
    ever stalls on ACT in program order.
  - attnw accumulated over j by PE: lhsT = (1/Z) broadcast via a stride-0
    AP to 64 columns, two heads col-packed ((0,0)/(0,64)), PSUM-accumulated
    across the 8 j-tiles.
  - applied^T = V^T * attnw_bcast (DVE), then the output projection runs in
    two 4-chunk waves through the attnw PSUM slot (no extra PSUM pool; PSUM
    is exactly 8 banks: 3 x score-slot (2) + attnw/out-wave slot (2)).
  - All tile pools are created once and shared across timing-loop bodies
    (weights/qkv double-buffered) so consecutive bodies overlap; timing
    builds unroll 4 bodies per For_i trip to amortize Tile's per-trip
    drain/barrier.
"""
import numpy as np
from contextlib import ExitStack

import concourse.bass as bass
import concourse.mybir as mybir
import concourse.tile as tile
from concourse import bacc
from concourse.vector_clock import ScopedClock
from concourse.bass_utils import run_bass_kernel_spmd
import bass_rust

N_CORES = 8
B, N, C, H, D, O = 8, 1024, 256, 8, 64, 256
HD = H * D  # 512
FP32 = mybir.dt.float32
F16 = mybir.dt.float16
EXP = mybir.ActivationFunctionType.Exp

# Per-head-pair jt indices whose b-half exp reads PSUM directly on ACT
# (balances the DVE bottleneck against ACT's slack).
DIRECT_JTS = [(1, 3, 5, 7), (), (1, 3, 5, 7), ()]

# colsum pipeline lag behind scores, in jt steps (2 measured best on HW)
LAG = 2

_MAXW = 1  # max sync waits this toolchain's walrus accepts per instruction


class _TC(tile.TileContext):
    """TileContext that splits semaphore waits one-per-instruction.

    The walrus build in this toolchain rejects any instruction carrying more
    than one sync wait ("Too many sync wait commands"), while Tile's
    add_semaphores attaches all needed waits to the consuming instruction.
    Engines execute in order, so moving excess waits onto same-engine NOPs
    emitted immediately before the instruction is semantically identical.
    """

    def _commit_instruction(self, inst, lazy_reg_writes: bool = True):
        si = inst.sync_info
        if (
            si is not None
            and si.on_wait
            and len(si.on_wait) > _MAXW
            and inst.engine != mybir.EngineType.Unassigned
        ):
            waits = list(si.on_wait)
            inst.sync_info = bass_rust.SyncInfo(
                on_wait=waits[-_MAXW:], on_update=list(si.on_update or [])
            )
            for i in range(0, len(waits) - _MAXW, _MAXW):
                nop = self.nc.engines[inst.engine].nop(nofuse=True, hint="waitsplit")
                nop.ins.sync_info = bass_rust.SyncInfo(
                    on_wait=waits[i : i + _MAXW], on_update=[]
                )
        return super()._commit_instruction(inst, lazy_reg_writes)

    def _drain_and_barrier(self, tick_clock, wait_clock):
        probe = self.nc.sync.drain()
        wait_clock.add_sem_waits(
            probe.ins, ScopedClock({None: tick_clock.global_clock})
        )
        si = probe.ins.sync_info
        waits = list(si.on_wait or []) if si is not None else []
        if len(waits) > 1:
            probe.ins.sync_info = bass_rust.SyncInfo(
                on_wait=waits[:1], on_update=list(si.on_update or [])
            )
            for i in range(1, len(waits)):
                d = self.nc.sync.drain()
                d.ins.sync_info = bass_rust.SyncInfo(
                    on_wait=waits[i : i + 1], on_update=[]
                )
        self.nc.all_engine_barrier()
        assert self.sems is not None
        popped = self.nc._tile_sem_poison_stack.pop()
        assert popped is self._sem_poison
        self.nc.clear_and_free_semaphores(list(self.sems.allocated().values()))
        self.nc.all_engine_barrier()


def _bcast64(col_ap):
    """[P,1] AP -> [P,64] AP reading the same element 64x (free step 0)."""
    return bass.AP(col_ap.tensor, col_ap.offset, [list(col_ap.ap[0]), [0, 64]])


def _r(ap):
    """View an fp32 AP as float32r: same bits, full-rate PE matmul."""
    return ap.bitcast(mybir.dt.float32r)


def _emit_body(tc, pools, xt, wqkv, wot, out):
    nc = tc.nc
    wpool, qkvpool, gpool, stpool, rzpool, zpool, obpool, sps, awps = pools

    XT = [[None, None], [None, None]]
    WQC = [[None, None], [None, None], [None, None]]
    WOT = []

    def load_x(kc, ic):
        t = wpool.tile([128, 512], F16, tag=f"xt{kc}{ic}", name=f"xt{kc}{ic}")
        nc.sync.dma_start(
            t[:], xt[kc * 128 : (kc + 1) * 128, ic * 512 : (ic + 1) * 512]
        )
        XT[kc][ic] = t

    def load_w(col, kc):
        w = wpool.tile([128, HD], F16, tag=f"w{col}{kc}", name=f"w{col}{kc}")
        nc.sync.dma_start(
            w[:], wqkv[kc * 128 : (kc + 1) * 128, col * HD : (col + 1) * HD]
        )
        WQC[col][kc] = w

    load_x(0, 0); load_x(1, 0); load_w(0, 0); load_w(0, 1)
    load_x(0, 1); load_x(1, 1); load_w(1, 0); load_w(1, 1)
    load_w(2, 0); load_w(2, 1)
    for kt in range(4):
        w = wpool.tile([128, O], F16, tag=f"wot{kt}", name=f"wot{kt}")
        nc.sync.dma_start(w[:], wot[kt * 128 : (kt + 1) * 128, :])
        WOT.append(w)

    QT = [qkvpool.tile([128, N], F16, tag=f"q{m}", name=f"q{m}") for m in range(4)]
    KT = [qkvpool.tile([128, N], F16, tag=f"k{m}", name=f"k{m}") for m in range(4)]
    VT = [qkvpool.tile([128, N], F16, tag=f"v{m}", name=f"v{m}") for m in range(4)]
    APP = [qkvpool.tile([128, N], F16, tag=f"app{m}", name=f"app{m}") for m in range(4)]

    def project(col, m, dst):
        ps = sps.tile([128, N], FP32, tag="s")
        for ic in range(2):
            for kc in range(2):
                nc.tensor.matmul(
                    ps[:, ic * 512 : (ic + 1) * 512],
                    WQC[col][kc][:, m * 128 : (m + 1) * 128],
                    XT[kc][ic][:],
                    start=(kc == 0),
                    stop=(kc == 1),
                )
        with nc.allow_low_precision(reason="f16 activations"):
            nc.vector.tensor_copy(dst[:], ps[:])

    QKV = (QT, KT, VT)
    project(0, 0, QT[0])
    project(1, 0, KT[0])
    project(2, 0, VT[0])
    for t in range(4):
        aw = awps.tile([128, N], FP32, tag="aw")
        z = zpool.tile([128, 16], FP32, tag="z")
        pending = []  # jts whose recip+colsum are not yet emitted (depth LAG)

        def colsum(ga, gb, rz, jt):
            for ic in range(2):
                icsl = slice(ic * 512, (ic + 1) * 512)
                nc.tensor.matmul(
                    aw[0:64, icsl], _bcast64(rz[:, 0:1]), ga[:, icsl],
                    start=(jt == 0), stop=(jt == 7),
                    tile_position=(0, 0), skip_group_check=True,
                )
                nc.tensor.matmul(
                    aw[64:128, icsl], _bcast64(rz[:, 1:2]), gb[:, icsl],
                    start=(jt == 0), stop=(jt == 7),
                    tile_position=(0, 64), skip_group_check=True,
                )

        def recip(jt):
            rz = rzpool.tile([128, 2], F16, tag="rz")
            with nc.allow_low_precision(reason="f16 matmul lhsT"):
                nc.vector.reciprocal(rz[:], z[:, 2 * jt : 2 * jt + 2])
            return rz

        for jt in range(8):
            jsl = slice(jt * 128, (jt + 1) * 128)
            sa = sps.tile([128, N], FP32, tag="s")
            sb_ = sps.tile([128, N], FP32, tag="s")
            # sa's two chunks first so its DVE bounce starts one matmul sooner
            for ic in range(2):
                icsl = slice(ic * 512, (ic + 1) * 512)
                nc.tensor.matmul(
                    sa[:, icsl], KT[t][0:64, jsl], QT[t][0:64, icsl],
                    start=True, stop=True,
                )
            for ic in range(2):
                icsl = slice(ic * 512, (ic + 1) * 512)
                nc.tensor.matmul(
                    sb_[:, icsl], KT[t][64:128, jsl], QT[t][64:128, icsl],
                    start=True, stop=True, tile_position=(64, 0),
                )
            ga = gpool.tile([128, N], F16, tag="g")
            gb = gpool.tile([128, N], F16, tag="g")
            for half, sps_tile, g_tile in ((0, sa, ga), (1, sb_, gb)):
                direct = half == 1 and jt in DIRECT_JTS[t]
                if direct:
                    src = sps_tile
                else:
                    st = stpool.tile([128, N], F16, tag="st", name="st")
                    nc.vector.tensor_copy(st[:], sps_tile[:])
                    src = st
                nc.scalar.activation(
                    g_tile[:], src[:], EXP, scale=0.125,
                    accum_out=z[:, 2 * jt + half : 2 * jt + half + 1],
                )
            pending.append((ga, gb, jt))
            if len(pending) > LAG:
                pga, pgb, pjt = pending.pop(0)
                colsum(pga, pgb, recip(pjt), pjt)
            if t < 3 and jt >= 5:
                col = jt - 5
                project(col, t + 1, QKV[col][t + 1])
        for pga, pgb, pjt in pending:
            colsum(pga, pgb, recip(pjt), pjt)
        with nc.allow_low_precision(reason="f16 activations"):
            nc.vector.tensor_mul(APP[t][:], VT[t][:], aw[:])

    # Output projection in two 4-chunk waves through the aw PSUM slot (same
    # tag+shape as aw, so no extra PSUM pool; naturally ordered after the
    # last APP-mul has consumed aw).
    for w in range(2):
        po = awps.tile([128, N], FP32, tag="aw")
        for sub in range(4):
            it = w * 4 + sub
            itsl = slice(it * 128, (it + 1) * 128)
            for kt in range(4):
                nc.tensor.matmul(
                    po[:, sub * O : (sub + 1) * O], APP[kt][:, itsl], WOT[kt][:],
                    start=(kt == 0), stop=(kt == 3),
                )
        ob = obpool.tile([128, N], FP32, tag="ob")
        nc.vector.tensor_copy(ob[:], po[:])
        # ob partition p, flat free (it 4, col 256) -> out rows w*512+it*128+p
        dst = bass.AP(
            out.tensor,
            out.offset + (w * 512) * O,
            [[O, 128], [128 * O, 4], [1, O]],
        )
        nc.sync.dma_start(dst, ob[:])


def build_nc(loop=0, unroll=4, use_bacc=False):
    """loop=0: single body (the graded kernel). loop=L: L body executions
    for timing, emitted as For_i(L // unroll) trips x unroll bodies (plus a
    straight-line remainder) to amortize Tile's per-trip drain/barrier."""
    cls = bacc.Bacc if use_bacc else bass.Bass
    nc = cls("TRN2", target_bir_lowering=False, debug=False, num_devices=N_CORES)
    xt = nc.declare_dram_parameter("xt", [C, N], F16, isOutput=False)
    wqkv = nc.declare_dram_parameter("wqkv", [C, 3 * HD], F16, isOutput=False)
    wot = nc.declare_dram_parameter("wot", [HD, O], F16, isOutput=False)
    out = nc.declare_dram_parameter("out", [N, O], FP32, isOutput=True)
    with _TC(nc, num_cores=N_CORES) as tc:
        with ExitStack() as ctx:
            pools = (
                ctx.enter_context(tc.tile_pool(name="w", bufs=2)),
                ctx.enter_context(tc.tile_pool(name="qkv", bufs=2)),
                ctx.enter_context(tc.tile_pool(name="g", bufs=8)),
                ctx.enter_context(tc.tile_pool(name="st", bufs=4)),
                ctx.enter_context(tc.tile_pool(name="rz", bufs=4)),
                ctx.enter_context(tc.tile_pool(name="z", bufs=2)),
                ctx.enter_context(tc.tile_pool(name="ob", bufs=2)),
                ctx.enter_context(tc.tile_pool(name="sps", bufs=3, space="PSUM")),
                ctx.enter_context(tc.tile_pool(name="awps", bufs=1, space="PSUM")),
            )
            args = (tc, pools, xt.ap(), wqkv.ap(), wot.ap(), out.ap())
            trips, rem = divmod(loop, unroll)
            if loop == 0:
                _emit_body(*args)
            else:
                if trips:
                    with tc.For_i(0, trips, 1):
                        for _ in range(unroll):
                            _emit_body(*args)
                for _ in range(rem):
                    _emit_body(*args)
    return nc


def make_in_maps(features, weight_q, weight_k, weight_v, weight_out):
    wqkv = np.ascontiguousarray(
        np.concatenate(
            [
                weight_q.reshape(C, HD),
                weight_k.reshape(C, HD),
                weight_v.reshape(C, HD),
            ],
            axis=1,
        ),
        dtype=np.float16,
    )
    wot = np.ascontiguousarray(weight_out.reshape(O, HD).T, dtype=np.float16)
    in_maps = []
    for b in range(B):
        xt = np.ascontiguousarray(features[b].T, dtype=np.float16)
        in_maps.append({"xt": xt, "wqkv": wqkv, "wot": wot})
    return in_maps


_CACHED_NC = None


def kernel(features, weight_q, weight_k, weight_v, weight_out):
    global _CACHED_NC
    if _CACHED_NC is None:
        _CACHED_NC = build_nc(loop=0)
    in_maps = make_in_maps(
        np.asarray(features, np.float32),
        np.asarray(weight_q, np.float32),
        np.asarray(weight_k, np.float32),
        np.asarray(weight_v, np.float32),
        np.asarray(weight_out, np.float32),
    )
    res = run_bass_kernel_spmd(_CACHED_NC, in_maps, list(range(N_CORES)))
    return np.stack([res.results[b]["out"] for b in range(B)], axis=0)


if __name__ == "__main__":
    rng = np.random.default_rng(0)
    feats = rng.standard_normal((B, N, C)).astype(np.float32)
    wq = rng.standard_normal((C, H, D)).astype(np.float32) * 0.05
    wk = rng.standard_normal((C, H, D)).astype(np.float32) * 0.05
    wv = rng.standard_normal((C, H, D)).astype(np.float32) * 0.05
    wo = rng.standard_normal((O, H, D)).astype(np.float32) * 0.05
    o = kernel(feats, wq, wk, wv, wo)
    print("kernel ran, out shape", o.shape, "finite:", np.isfinite(o).all())


# revision 8
# speedup vs baseline: 1.3131x; 1.0137x over previous
"""Trainium2 Bass kernel for nn_MultiHeadAttention_62371515073076.

Math (per batch b, faithful to the reference's quirky softmax over the QUERY
axis):
  q/k/v = einsum('nc,chd->nhd', x, W{q,k,v})
  s[i,j,h] = q[i,h,:].k[j,h,:] / 8
  p = softmax over i  (query axis!)
  attnw[i,h] = sum_j p[i,j,h] = sum_j exp(s[i,j,h]) / Z[j,h],
               Z[j,h] = sum_i exp(s[i,j,h])
  out = einsum('ihd,ohd->io', v * attnw, Wout)

Sharding: batch 8 -> one batch per NeuronCore (data parallel), weights
replicated. Host pre-transposes x -> xt (C,N) and Wout -> wot (HD,O); the
q/k/v weights are fed natural (C,HD) concatenated (already the lhsT layout).

Per-core dataflow (engine-balanced against the measured TRN2 rates:
ACT 1 cyc/elem from SBUF but ~2 from PSUM; DVE ~1 cyc/elem any source;
GPSIMD and DMA cannot access PSUM at all, so every PSUM byte is drained by
DVE or ACT):

  - QKV projections -> Q^T/K^T/V^T [hd, i] f16 tiles (PE, K=128 matmuls;
    PSUM drained by DVE). Projections for head-pair t+1 are interleaved
    into t's j-loop so ACT never idles at pair boundaries.
  - Scores S^T [j, i] per head, two heads row-packed in the PE array
    (tile_position (0,0)/(64,0)).
  - exp via ACT with fused row-sum (accum_out) -> Z columns; most tiles are
    first bounced PSUM->SBUF f16 by DVE (ACT streams SBUF 2x faster than
    PSUM); a measured subset reads PSUM directly to use ACT's slack
    (DIRECT_JTS). No max-subtraction: |s| <~ 5 so fp32/f16 exp is safe.
  - The j-loop is software-pipelined: scores(jt) are emitted before
    colsum(jt-1) and recip(jt-1) after bounces(jt), so neither PE nor DVE
    ever stalls on ACT in program order.
  - attnw accumulated over j by PE: lhsT = (1/Z) broadcast via a stride-0
    AP to 64 columns, two heads col-packed ((0,0)/(0,64)), PSUM-accumulated
    across the 8 j-tiles.
  - applied^T = V^T * attnw_bcast (DVE), then the output projection runs in
    two 4-chunk waves through the attnw PSUM slot (no extra PSUM pool; PSUM
    is exactly 8 banks: 3 x score-slot (2) + attnw/out-wave slot (2)).
  - All tile pools are created once and shared across timing-loop bodies
    (weights/qkv double-buffered) so consecutive bodies overlap; timing
    builds unroll 4 bodies per For_i trip to amortize Tile's per-trip
    drain/barrier.
"""
import numpy as np
from contextlib import ExitStack

import concourse.bass as bass
import concourse.mybir as mybir
import concourse.tile as tile
from concourse import bacc
from concourse.vector_clock import ScopedClock
from concourse.bass_utils import run_bass_kernel_spmd
import bass_rust

N_CORES = 8
B, N, C, H, D, O = 8, 1024, 256, 8, 64, 256
HD = H * D  # 512
FP32 = mybir.dt.float32
F16 = mybir.dt.float16
EXP = mybir.ActivationFunctionType.Exp

# Per-head-pair jt indices whose b-half exp reads PSUM directly on ACT
# (balances the DVE bottleneck against ACT's slack).
DIRECT_JTS = [(1, 3, 5, 7), (1, 5), (1, 3, 5, 7), (1, 5)]

# colsum pipeline lag behind scores, in jt steps (2 measured best on HW)
LAG = 2

_MAXW = 1  # max sync waits this toolchain's walrus accepts per instruction


class _TC(tile.TileContext):
    """TileContext that splits semaphore waits one-per-instruction.

    The walrus build in this toolchain rejects any instruction carrying more
    than one sync wait ("Too many sync wait commands"), while Tile's
    add_semaphores attaches all needed waits to the consuming instruction.
    Engines execute in order, so moving excess waits onto same-engine NOPs
    emitted immediately before the instruction is semantically identical.
    """

    def _commit_instruction(self, inst, lazy_reg_writes: bool = True):
        si = inst.sync_info
        if (
            si is not None
            and si.on_wait
            and len(si.on_wait) > _MAXW
            and inst.engine != mybir.EngineType.Unassigned
        ):
            waits = list(si.on_wait)
            inst.sync_info = bass_rust.SyncInfo(
                on_wait=waits[-_MAXW:], on_update=list(si.on_update or [])
            )
            for i in range(0, len(waits) - _MAXW, _MAXW):
                nop = self.nc.engines[inst.engine].nop(nofuse=True, hint="waitsplit")
                nop.ins.sync_info = bass_rust.SyncInfo(
                    on_wait=waits[i : i + _MAXW], on_update=[]
                )
        return super()._commit_instruction(inst, lazy_reg_writes)

    def _drain_and_barrier(self, tick_clock, wait_clock):
        probe = self.nc.sync.drain()
        wait_clock.add_sem_waits(
            probe.ins, ScopedClock({None: tick_clock.global_clock})
        )
        si = probe.ins.sync_info
        waits = list(si.on_wait or []) if si is not None else []
        if len(waits) > 1:
            probe.ins.sync_info = bass_rust.SyncInfo(
                on_wait=waits[:1], on_update=list(si.on_update or [])
            )
            for i in range(1, len(waits)):
                d = self.nc.sync.drain()
                d.ins.sync_info = bass_rust.SyncInfo(
                    on_wait=waits[i : i + 1], on_update=[]
                )
        self.nc.all_engine_barrier()
        assert self.sems is not None
        popped = self.nc._tile_sem_poison_stack.pop()
        assert popped is self._sem_poison
        self.nc.clear_and_free_semaphores(list(self.sems.allocated().values()))
        self.nc.all_engine_barrier()


def _bcast64(col_ap):
    """[P,1] AP -> [P,64] AP reading the same element 64x (free step 0)."""
    return bass.AP(col_ap.tensor, col_ap.offset, [list(col_ap.ap[0]), [0, 64]])


def _r(ap):
    """View an fp32 AP as float32r: same bits, full-rate PE matmul."""
    return ap.bitcast(mybir.dt.float32r)


def _emit_body(tc, pools, xt, wqkv, wot, out):
    nc = tc.nc
    wpool, qkvpool, gpool, stpool, rzpool, zpool, obpool, sps, awps = pools

    XT = [[None, None], [None, None]]
    WQC = [[None, None], [None, None], [None, None]]
    WOT = []

    def load_x(kc, ic):
        t = wpool.tile([128, 512], F16, tag=f"xt{kc}{ic}", name=f"xt{kc}{ic}")
        nc.sync.dma_start(
            t[:], xt[kc * 128 : (kc + 1) * 128, ic * 512 : (ic + 1) * 512]
        )
        XT[kc][ic] = t

    def load_w(col, kc):
        w = wpool.tile([128, HD], F16, tag=f"w{col}{kc}", name=f"w{col}{kc}")
        nc.sync.dma_start(
            w[:], wqkv[kc * 128 : (kc + 1) * 128, col * HD : (col + 1) * HD]
        )
        WQC[col][kc] = w

    load_x(0, 0); load_x(1, 0); load_w(0, 0); load_w(0, 1)
    load_x(0, 1); load_x(1, 1); load_w(1, 0); load_w(1, 1)
    load_w(2, 0); load_w(2, 1)
    for kt in range(4):
        w = wpool.tile([128, O], F16, tag=f"wot{kt}", name=f"wot{kt}")
        nc.sync.dma_start(w[:], wot[kt * 128 : (kt + 1) * 128, :])
        WOT.append(w)

    QT = [qkvpool.tile([128, N], F16, tag=f"q{m}", name=f"q{m}") for m in range(4)]
    KT = [qkvpool.tile([128, N], F16, tag=f"k{m}", name=f"k{m}") for m in range(4)]
    VT = [qkvpool.tile([128, N], F16, tag=f"v{m}", name=f"v{m}") for m in range(4)]
    APP = [qkvpool.tile([128, N], F16, tag=f"app{m}", name=f"app{m}") for m in range(4)]

    def project(col, m, dst):
        ps = sps.tile([128, N], FP32, tag="s")
        for ic in range(2):
            for kc in range(2):
                nc.tensor.matmul(
                    ps[:, ic * 512 : (ic + 1) * 512],
                    WQC[col][kc][:, m * 128 : (m + 1) * 128],
                    XT[kc][ic][:],
                    start=(kc == 0),
                    stop=(kc == 1),
                )
        with nc.allow_low_precision(reason="f16 activations"):
            nc.vector.tensor_copy(dst[:], ps[:])

    QKV = (QT, KT, VT)
    project(0, 0, QT[0])
    project(1, 0, KT[0])
    project(2, 0, VT[0])
    for t in range(4):
        aw = awps.tile([128, N], FP32, tag="aw")
        z = zpool.tile([128, 16], FP32, tag="z")
        pending = []  # jts whose recip+colsum are not yet emitted (depth LAG)

        def colsum(ga, gb, rz, jt):
            for ic in range(2):
                icsl = slice(ic * 512, (ic + 1) * 512)
                nc.tensor.matmul(
                    aw[0:64, icsl], _bcast64(rz[:, 0:1]), ga[:, icsl],
                    start=(jt == 0), stop=(jt == 7),
                    tile_position=(0, 0), skip_group_check=True,
                )
                nc.tensor.matmul(
                    aw[64:128, icsl], _bcast64(rz[:, 1:2]), gb[:, icsl],
                    start=(jt == 0), stop=(jt == 7),
                    tile_position=(0, 64), skip_group_check=True,
                )

        def recip(jt):
            rz = rzpool.tile([128, 2], F16, tag="rz")
            with nc.allow_low_precision(reason="f16 matmul lhsT"):
                nc.vector.reciprocal(rz[:], z[:, 2 * jt : 2 * jt + 2])
            return rz

        for jt in range(8):
            jsl = slice(jt * 128, (jt + 1) * 128)
            sa = sps.tile([128, N], FP32, tag="s")
            sb_ = sps.tile([128, N], FP32, tag="s")
            # sa's two chunks first so its DVE bounce starts one matmul sooner
            for ic in range(2):
                icsl = slice(ic * 512, (ic + 1) * 512)
                nc.tensor.matmul(
                    sa[:, icsl], KT[t][0:64, jsl], QT[t][0:64, icsl],
                    start=True, stop=True,
                )
            for ic in range(2):
                icsl = slice(ic * 512, (ic + 1) * 512)
                nc.tensor.matmul(
                    sb_[:, icsl], KT[t][64:128, jsl], QT[t][64:128, icsl],
                    start=True, stop=True, tile_position=(64, 0),
                )
            ga = gpool.tile([128, N], F16, tag="g")
            gb = gpool.tile([128, N], F16, tag="g")
            for half, sps_tile, g_tile in ((0, sa, ga), (1, sb_, gb)):
                direct = half == 1 and jt in DIRECT_JTS[t]
                if direct:
                    src = sps_tile
                else:
                    st = stpool.tile([128, N], F16, tag="st", name="st")
                    nc.vector.tensor_copy(st[:], sps_tile[:])
                    src = st
                nc.scalar.activation(
                    g_tile[:], src[:], EXP, scale=0.125,
                    accum_out=z[:, 2 * jt + half : 2 * jt + half + 1],
                )
            pending.append((ga, gb, jt))
            if len(pending) > LAG:
                pga, pgb, pjt = pending.pop(0)
                colsum(pga, pgb, recip(pjt), pjt)
            if t < 3 and jt >= 5:
                col = jt - 5
                project(col, t + 1, QKV[col][t + 1])
        for pga, pgb, pjt in pending:
            colsum(pga, pgb, recip(pjt), pjt)
        with nc.allow_low_precision(reason="f16 activations"):
            nc.vector.tensor_mul(APP[t][:], VT[t][:], aw[:])

    # Output projection in two 4-chunk waves through the aw PSUM slot (same
    # tag+shape as aw, so no extra PSUM pool; naturally ordered after the
    # last APP-mul has consumed aw).
    for w in range(2):
        po = awps.tile([128, N], FP32, tag="aw")
        for sub in range(4):
            it = w * 4 + sub
            itsl = slice(it * 128, (it + 1) * 128)
            for kt in range(4):
                nc.tensor.matmul(
                    po[:, sub * O : (sub + 1) * O], APP[kt][:, itsl], WOT[kt][:],
                    start=(kt == 0), stop=(kt == 3),
                )
        ob = obpool.tile([128, N], FP32, tag="ob")
        nc.vector.tensor_copy(ob[:], po[:])
        # ob partition p, flat free (it 4, col 256) -> out rows w*512+it*128+p
        dst = bass.AP(
            out.tensor,
            out.offset + (w * 512) * O,
            [[O, 128], [128 * O, 4], [1, O]],
        )
        nc.sync.dma_start(dst, ob[:])


def build_nc(loop=0, unroll=4, use_bacc=False):
    """loop=0: single body (the graded kernel). loop=L: L body executions
    for timing, emitted as For_i(L // unroll) trips x unroll bodies (plus a
    straight-line remainder) to amortize Tile's per-trip drain/barrier."""
    cls = bacc.Bacc if use_bacc else bass.Bass
    nc = cls("TRN2", target_bir_lowering=False, debug=False, num_devices=N_CORES)
    xt = nc.declare_dram_parameter("xt", [C, N], F16, isOutput=False)
    wqkv = nc.declare_dram_parameter("wqkv", [C, 3 * HD], F16, isOutput=False)
    wot = nc.declare_dram_parameter("wot", [HD, O], F16, isOutput=False)
    out = nc.declare_dram_parameter("out", [N, O], FP32, isOutput=True)
    with _TC(nc, num_cores=N_CORES) as tc:
        with ExitStack() as ctx:
            pools = (
                ctx.enter_context(tc.tile_pool(name="w", bufs=2)),
                ctx.enter_context(tc.tile_pool(name="qkv", bufs=2)),
                ctx.enter_context(tc.tile_pool(name="g", bufs=8)),
                ctx.enter_context(tc.tile_pool(name="st", bufs=4)),
                ctx.enter_context(tc.tile_pool(name="rz", bufs=4)),
                ctx.enter_context(tc.tile_pool(name="z", bufs=2)),
                ctx.enter_context(tc.tile_pool(name="ob", bufs=2)),
                ctx.enter_context(tc.tile_pool(name="sps", bufs=3, space="PSUM")),
                ctx.enter_context(tc.tile_pool(name="awps", bufs=1, space="PSUM")),
            )
            args = (tc, pools, xt.ap(), wqkv.ap(), wot.ap(), out.ap())
            trips, rem = divmod(loop, unroll)
            if loop == 0:
                _emit_body(*args)
            else:
                if trips:
                    with tc.For_i(0, trips, 1):
                        for _ in range(unroll):
                            _emit_body(*args)
                for _ in range(rem):
                    _emit_body(*args)
    return nc


def make_in_maps(features, weight_q, weight_k, weight_v, weight_out):
    wqkv = np.ascontiguousarray(
        np.concatenate(
            [
                weight_q.reshape(C, HD),
                weight_k.reshape(C, HD),
                weight_v.reshape(C, HD),
            ],
            axis=1,
        ),
        dtype=np.float16,
    )
    wot = np.ascontiguousarray(weight_out.reshape(O, HD).T, dtype=np.float16)
    in_maps = []
    for b in range(B):
        xt = np.ascontiguousarray(features[b].T, dtype=np.float16)
        in_maps.append({"xt": xt, "wqkv": wqkv, "wot": wot})
    return in_maps


_CACHED_NC = None


def kernel(features, weight_q, weight_k, weight_v, weight_out):
    global _CACHED_NC
    if _CACHED_NC is None:
        _CACHED_NC = build_nc(loop=0)
    in_maps = make_in_maps(
        np.asarray(features, np.float32),
        np.asarray(weight_q, np.float32),
        np.asarray(weight_k, np.float32),
        np.asarray(weight_v, np.float32),
        np.asarray(weight_out, np.float32),
    )
    res = run_bass_kernel_spmd(_CACHED_NC, in_maps, list(range(N_CORES)))
    return np.stack([res.results[b]["out"] for b in range(B)], axis=0)


if __name__ == "__main__":
    rng = np.random.default_rng(0)
    feats = rng.standard_normal((B, N, C)).astype(np.float32)
    wq = rng.standard_normal((C, H, D)).astype(np.float32) * 0.05
    wk = rng.standard_normal((C, H, D)).astype(np.float32) * 0.05
    wv = rng.standard_normal((C, H, D)).astype(np.float32) * 0.05
    wo = rng.standard_normal((O, H, D)).astype(np.float32) * 0.05
    o = kernel(feats, wq, wk, wv, wo)
    print("kernel ran, out shape", o.shape, "finite:", np.isfinite(o).all())
